# revision 23
# baseline (speedup 1.0000x reference)
"""Trainium2 Bass kernel for nn_NASAdapter (GDAS single-edge cell).

The Gumbel straight-through gate (10 logits) is computed on host; only the
selected branch runs on device. With the problem's fixed inputs the argmax
selects branch 8 = dil_conv_7 (depthwise dilated conv k=7 -> pointwise
768x768 conv -> BatchNorm), followed by the GDAS weighted residual.

Sharding: data-parallel over batch, one batch element per NeuronCore (8x).
BatchNorm statistics span the whole batch, so per-core partial stats are
combined with a tiny in-kernel AllReduce.

Device layout notes:
  - Host pre-transposes inputs to channel-major and pre-transposes the
    pointwise weight so the device never transposes operands.
  - Pointwise conv: out[co,s] += wp_T[ci,co].T @ y1[ci,s] over 6 ci-blocks,
    PSUM-accumulated, 36 matmuls of N=512 per core.
  - Depthwise dilated conv: 7 fused multiply-add taps per channel block on
    the vector engine (scalar_tensor_tensor with per-partition tap weight).
  - Output is transposed back to [s, h] with PE transposes.

Compiler constraint handled throughout: on this toolchain every compute
instruction may carry at most ONE semaphore wait.  Tile emits one wait per
distinct unobserved producer clock (engine sems and 8 DMA-lane sems), so
tiny "observer" ops make each engine observe one DMA lane at a time before
any real op that would otherwise need two waits.
"""

import sys

if "/opt/trn_rl_repo" not in sys.path:
    sys.path.insert(0, "/opt/trn_rl_repo")

import numpy as np

B, S, H = 8, 512, 768
P = 128
NB = H // P  # 6 channel blocks
N_CORES = 8
EPS = 1e-5
TEM = 10.0

_f32 = np.float32


# ----------------------------------------------------------------- host gate
def _gate(u: np.ndarray, arch_parameters: np.ndarray):
    u = u.astype(_f32)
    ap = arch_parameters.astype(_f32)
    uc = np.clip(u, _f32(1e-9), _f32(1.0 - 1e-9))
    gumbels = -np.log(-np.log(uc))
    m = ap.max(axis=1, keepdims=True)
    ls = ap - m - np.log(np.sum(np.exp(ap - m), axis=1, keepdims=True))
    logits = ((ls + gumbels) / _f32(TEM)).astype(_f32)
    lm = logits.max(axis=1, keepdims=True)
    e = np.exp(logits - lm)
    probs = (e / e.sum(axis=1, keepdims=True)).astype(_f32)
    idx = int(np.argmax(probs, axis=-1)[0])
    one_h = np.zeros_like(probs)
    one_h[0, idx] = 1.0
    hardwts = ((one_h - probs) + probs).astype(_f32)
    w_sel = _f32(hardwts[0, idx])
    c_add = _f32(np.sum(hardwts, dtype=_f32) - w_sel)
    return idx, w_sel, c_add


# ------------------------------------------------------- device kernel build
_BUILD_CACHE = {}
_DRAIN_PATCHED = False


def _patch_tile_drain():
    """This toolchain's walrus encodes at most ONE semaphore wait per
    instruction (single NEURON_ISA_TPB_EVENTS slot) and refuses multi-wait
    instructions instead of splitting them.  Tile's kernel-tail drain
    accumulates one wait per outstanding clock (engines + DMA lanes +
    collectives), so split it: keep one wait on the drain and emit one
    single-wait NoOp per remaining clock before the barrier."""
    global _DRAIN_PATCHED
    if _DRAIN_PATCHED:
        return
    from concourse.tile import TileContext
    from concourse.vector_clock import ScopedClock
    from concourse import mybir

    def _drain_and_barrier(self, tick_clock, wait_clock):
        nc = self.nc
        drain_inst = nc.sync.drain()
        wait_clock.add_sem_waits(
            drain_inst.ins, ScopedClock({None: tick_clock.global_clock})
        )
        si = drain_inst.ins.sync_info
        if si is not None and len(si.on_wait) > 1:
            waits = list(si.on_wait)
            drain_inst.ins.sync_info = mybir.SyncInfo(
                on_wait=[waits[0]], on_update=list(si.on_update)
            )
            for w in waits[1:]:
                nop = nc.sync.nop(hint="drain_wait_split", nofuse=True)
                nop.ins.sync_info = mybir.SyncInfo(on_wait=[w], on_update=[])

        nc.all_engine_barrier()
        assert self.sems is not None
        popped = nc._tile_sem_poison_stack.pop()
        assert popped is self._sem_poison
        nc.clear_and_free_semaphores(list(self.sems.allocated().values()))
        nc.all_engine_barrier()

    TileContext._drain_and_barrier = _drain_and_barrier
    _DRAIN_PATCHED = True


def _build_dil_kernel(K: int, ablate=(), split=(4, 1, 1)):
    """dil_conv_K: depthwise (K taps, dilation 2, pad K-1) -> pointwise ->
    BN -> w_sel*. + const + residual.

    split = (#blocks on DVE, #blocks on GpSimd, #blocks on PE) for the
    depthwise conv; must sum to NB=6.

    v3: bf16 conv datapath with fp32 depthwise accumulators (last tap writes
    bf16), co-outer matmuls, AllGather stats exchange with local reduction,
    BN scale+bias fused into the PSUM->SBUF copy via Identity activation,
    residual added post-transpose from natural-layout fp32 x.

    ablate: subset of {"dw","mm","cc","tpose"} for timing experiments only.
    """
    from concourse.bass import Bass
    from concourse.tile import TileContext
    from concourse import mybir, masks

    _patch_tile_drain()

    F32 = mybir.dt.float32
    BF16 = mybir.dt.bfloat16
    AF = mybir.ActivationFunctionType
    OP = mybir.AluOpType

    SP = S + 2 * (K - 1)  # padded length for the dilated depthwise conv
    NSB = S // P          # 4 sequence blocks
    n_dve, n_pool, n_pe = split
    assert n_dve + n_pool + n_pe == NB
    DVE_BLOCKS = tuple(range(n_dve))
    POOL_BLOCKS = tuple(range(n_dve, n_dve + n_pool))
    PE_BLOCKS = tuple(range(n_dve + n_pool, NB))

    nc = Bass(num_devices=N_CORES)
    xr_in = nc.dram_tensor("xr_pad", [P, NB, SP], BF16, kind="ExternalInput")
    xn_in = nc.dram_tensor("x_nat", [P, NSB, H], F32, kind="ExternalInput")
    # wd (NB*K) and gamma/beta (2*NB) merged into one tensor = one DMA lane
    wg_in = nc.dram_tensor("wg_pk", [P, NB * K + 2 * NB], F32, kind="ExternalInput")
    wp_in = nc.dram_tensor("wp_t", [P, NB, H], BF16, kind="ExternalInput")
    # diagonalized depthwise taps for the PE blocks
    wdg_in = nc.dram_tensor("wdiag", [P, max(n_pe, 1), K, P], BF16, kind="ExternalInput")
    out_t = nc.dram_tensor("out", [S, H], F32, kind="ExternalOutput")
    cc_in = nc.dram_tensor("cc_in", [P, 2 * NB], F32)
    cc_out = nc.dram_tensor("cc_out", [N_CORES, P, 2 * NB], F32, addr_space="Shared")

    with TileContext(nc) as tc:
        with (
            tc.tile_pool(name="big", bufs=1) as big,
            tc.tile_pool(name="small", bufs=1) as small,
            tc.tile_pool(name="obs", bufs=8) as obs,
            tc.tile_pool(name="psy", bufs=1, space="PSUM") as psy_pool,
            tc.tile_pool(name="pst", bufs=2, space="PSUM") as pst_pool,
        ):
            # ---- input DMAs (one per tensor => one DMA lane each)
            xr = big.tile([P, NB, SP], BF16, tag="xr")
            nc.sync.dma_start(out=xr, in_=xr_in[:, :, :])
            xn = big.tile([P, NSB, H], F32, tag="xn")
            nc.sync.dma_start(out=xn, in_=xn_in[:, :, :])
            wp = big.tile([P, NB, H], BF16, tag="wp")
            nc.sync.dma_start(out=wp, in_=wp_in[:, :, :])
            wg = small.tile([P, NB * K + 2 * NB], F32, tag="wg")
            nc.sync.dma_start(out=wg, in_=wg_in[:, :])
            wd = wg[:, 0 : NB * K]
            gb = wg[:, NB * K : NB * K + 2 * NB]
            if n_pe:
                wdg = big.tile([P, n_pe, K, P], BF16, tag="wdg")
                nc.sync.dma_start(out=wdg, in_=wdg_in[:, 0:n_pe, :, :])

            ident = small.tile([P, P], F32, tag="ident")
            masks.make_identity(nc, ident[:, :])

            # ---- observers: each engine observes one DMA lane per op
            o1 = obs.tile([P, 1], F32, tag="o")
            nc.vector.tensor_copy(out=o1, in_=wg[:, 0:1])
            o3 = obs.tile([P, 1], F32, tag="o")
            nc.vector.tensor_copy(out=o3, in_=xn[:, 0, 0:1])
            if n_pool:
                o4 = obs.tile([P, 1], F32, tag="o")
                nc.gpsimd.tensor_copy(out=o4, in_=wg[:, 0:1])
                o5 = obs.tile([P, 1], BF16, tag="o5")
                nc.gpsimd.tensor_copy(out=o5, in_=xr[:, 0, 0:1])

            # ---- depthwise dilated conv: y1[c,s] = sum_j wd[c,j]*xr[c,s+2j]
            # fp32 accumulators; the last tap writes the bf16 matmul input.
            # DVE uses fused STT; GpSimd has no STT ucode -> TS+TT pairs;
            # PE blocks use diagonalized weight matmuls into PSUM.
            y1 = [None] * NB    # bf16 matmul inputs
            acc = [None] * NB   # fp32 accumulators
            ptmp = [None] * NB
            one_tap = "dw" in ablate
            for b in range(NB):
                eng = nc.vector if b in DVE_BLOCKS else nc.gpsimd
                y1[b] = big.tile([P, S], BF16, tag=f"y1_{b}", name=f"y1_{b}")
                if b in PE_BLOCKS:
                    continue
                if one_tap or K == 1:
                    eng.tensor_scalar(
                        out=y1[b], in0=xr[:, b, 0:S], scalar1=wd[:, b * K : b * K + 1],
                        scalar2=None, op0=OP.mult,
                    )
                else:
                    acc[b] = big.tile([P, S], F32, tag=f"acc_{b}", name=f"acc_{b}")
                    eng.tensor_scalar(
                        out=acc[b], in0=xr[:, b, 0:S], scalar1=wd[:, b * K : b * K + 1],
                        scalar2=None, op0=OP.mult,
                    )
            for j in range(1, K):
                if one_tap:
                    break
                for b in range(NB):
                    if b in PE_BLOCKS:
                        continue
                    last = j == K - 1
                    dst = y1[b] if last else acc[b]
                    if b in DVE_BLOCKS:
                        nc.vector.scalar_tensor_tensor(
                            out=dst, in0=xr[:, b, 2 * j : 2 * j + S],
                            scalar=wd[:, b * K + j : b * K + j + 1],
                            in1=acc[b], op0=OP.mult, op1=OP.add,
                        )
                    else:
                        if ptmp[b] is None:
                            ptmp[b] = big.tile(
                                [P, S], F32, tag=f"ptmp_{b}", name=f"ptmp_{b}"
                            )
                        nc.gpsimd.tensor_scalar(
                            out=ptmp[b], in0=xr[:, b, 2 * j : 2 * j + S],
                            scalar1=wd[:, b * K + j : b * K + j + 1],
                            scalar2=None, op0=OP.mult,
                        )
                        nc.gpsimd.tensor_tensor(
                            out=dst, in0=acc[b], in1=ptmp[b], op=OP.add
                        )

            # ---- PE observers — write into scratch pst tile / psy[0] column,
            # harmlessly overwritten later.
            psy = [psy_pool.tile([P, S], F32, tag=f"psy_{b}", name=f"psy_{b}") for b in range(NB)]
            ot = pst_pool.tile([P, P], F32, tag="pt", name="obs_tpose")
            nc.tensor.transpose(ot, ident, ident)
            nc.tensor.matmul(
                psy[0][:, 0:1], wp[:, 0, 0:P], wp[:, 0, 0:1], start=True, stop=True
            )
            if n_pe:
                nc.tensor.matmul(
                    psy[0][:, 0:1], wdg[:, 0, 0, :], wdg[:, 0, 0, 0:1],
                    start=True, stop=True,
                )

            # PE depthwise blocks: 7 diag matmuls into PSUM, ACT copy to bf16
            for i, b in enumerate(PE_BLOCKS):
                yps = pst_pool.tile([P, S], F32, tag="pt", name=f"yps_{b}")
                taps = 1 if one_tap else K
                for j in range(taps):
                    nc.tensor.matmul(
                        yps,
                        wdg[:, i, j, :],
                        xr[:, b, 2 * j : 2 * j + S],
                        start=(j == 0),
                        stop=(j == taps - 1),
                    )
                nc.scalar.activation(out=y1[b], in_=yps, func=AF.Copy, scale=1.0)

            # ---- pointwise conv (bf16, co-outer so psy[co] completes early)
            # + per-core BN stats per channel as each psy[co] finishes
            exch = small.tile([P, 2 * NB], F32, tag="exch")
            nb_ci = 1 if "mm" in ablate else NB
            for co in range(NB):
                for ci in range(nb_ci):
                    nc.tensor.matmul(
                        psy[co],
                        wp[:, ci, co * P : (co + 1) * P],
                        y1[ci],
                        start=(ci == 0),
                        stop=(ci == nb_ci - 1),
                    )
                st = obs.tile([P, 6], F32, tag="bnst")
                nc.vector.bn_stats(out=st, in_=psy[co])
                mv = obs.tile([P, 2], F32, tag="bnmv")
                nc.vector.bn_aggr(out=mv, in_=st)
                nc.vector.tensor_copy(out=exch[:, co : co + 1], in_=mv[:, 0:1])
                nc.vector.scalar_tensor_tensor(
                    out=exch[:, NB + co : NB + co + 1], in0=mv[:, 0:1],
                    scalar=mv[:, 0:1], in1=mv[:, 1:2], op0=OP.mult, op1=OP.add,
                )

            # ---- cross-core stats exchange: AllGather + local sum
            # (AllGather is ~2x cheaper than AllReduce for latency-bound sizes)
            if "cc" in ablate:
                stats = exch
            else:
                nc.sync.dma_start(out=cc_in[:, :], in_=exch)
                nc.gpsimd.collective_compute(
                    "AllGather", OP.bypass,
                    replica_groups=[list(range(N_CORES))],
                    ins=[cc_in[:, :]], outs=[cc_out[:, :, :]],
                )
                allst = small.tile([P, N_CORES, 2 * NB], F32, tag="allst")
                nc.sync.dma_start(
                    out=allst, in_=cc_out.rearrange("r p c -> p r c")
                )
                stats = small.tile([P, 2 * NB], F32, tag="stats")
                nc.vector.tensor_tensor(
                    out=stats, in0=allst[:, 0, :], in1=allst[:, 1, :], op=OP.add
                )
                for r in range(2, N_CORES):
                    nc.vector.tensor_tensor(
                        out=stats, in0=stats, in1=allst[:, r, :], op=OP.add
                    )

            # ---- fold stats into per-channel scale/shift:
            #   gm = sum(mean)/8, e2 = sum(E[y^2])/8, var = e2 - gm^2
            #   A  = g' / sqrt(var+eps), Bc = b' - gm*A     (g',b' host-folded)
            gm = small.tile([P, NB], F32, tag="gm")
            nc.vector.tensor_scalar(
                out=gm, in0=stats[:, 0:NB], scalar1=1.0 / N_CORES,
                scalar2=None, op0=OP.mult,
            )
            var = small.tile([P, NB], F32, tag="var")
            nc.vector.tensor_scalar(
                out=var, in0=stats[:, NB : 2 * NB], scalar1=1.0 / N_CORES,
                scalar2=None, op0=OP.mult,
            )
            gm2 = small.tile([P, NB], F32, tag="gm2")
            nc.vector.tensor_tensor(out=gm2, in0=gm, in1=gm, op=OP.mult)
            nc.vector.tensor_tensor(out=var, in0=var, in1=gm2, op=OP.subtract)
            epsc = small.tile([P, 1], F32, tag="epsc")
            nc.vector.memset(epsc, EPS)
            sd = small.tile([P, NB], F32, tag="sd")
            nc.scalar.activation(out=sd, in_=var, func=AF.Sqrt, bias=epsc, scale=1.0)
            rstd = small.tile([P, NB], F32, tag="rstd")
            nc.vector.reciprocal(out=rstd, in_=sd)
            A = small.tile([P, NB], F32, tag="A")
            nc.vector.tensor_tensor(out=A, in0=gb[:, 0:NB], in1=rstd, op=OP.mult)
            Bc = small.tile([P, NB], F32, tag="Bc")
            gmA = small.tile([P, NB], F32, tag="gmA")
            nc.vector.tensor_tensor(out=gmA, in0=gm, in1=A, op=OP.mult)
            nc.vector.tensor_tensor(out=Bc, in0=gb[:, NB : 2 * NB], in1=gmA, op=OP.subtract)

            # ACT observer for the DVE-produced A/Bc before the BN-apply reads them
            oa = obs.tile([P, 1], F32, tag="oact")
            nc.scalar.activation(out=oa, in_=Bc[:, 0:1], func=AF.Copy, scale=1.0)

            # ---- fused BN apply: z = psy*A[b] + Bc[b]  (PSUM -> bf16 SBUF, 1 ACT op)
            zt = []
            for b in range(NB):
                z = big.tile([P, S], F32, tag=f"z_{b}", name=f"z_{b}")
                nc.scalar.activation(
                    out=z, in_=psy[b], func=AF.Identity,
                    scale=A[:, b : b + 1], bias=Bc[:, b : b + 1],
                )
                zt.append(z)

            # ---- transpose z blocks [co,s]->[s,co] (PE), then add the
            # residual from natural-layout fp32 x while copying to SBUF.
            # Dummy transpose of the LAST apply output first: PE observes the
            # ACT clock at its final tick once, so the real transposes carry
            # only their PSUM-slot WAR wait (single-wait rule).
            ot2 = pst_pool.tile([P, P], F32, tag="pt", name="obs_tpose2")
            nc.tensor.transpose(ot2, zt[NB - 1][:, 0:P], ident)
            out_all = big.tile([P, NSB, H], F32, tag="out_all")
            for sb in range(NSB):
                for b in range(NB):
                    if "tpose" in ablate:
                        nc.vector.tensor_copy(
                            out=out_all[:, sb, b * P : (b + 1) * P],
                            in_=zt[b][:, sb * P : (sb + 1) * P],
                        )
                        continue
                    pt = pst_pool.tile([P, P], F32, tag="pt")
                    nc.tensor.transpose(pt, zt[b][:, sb * P : (sb + 1) * P], ident)
                    nc.vector.tensor_tensor(
                        out=out_all[:, sb, b * P : (b + 1) * P],
                        in0=pt, in1=xn[:, sb, b * P : (b + 1) * P], op=OP.add,
                    )
            # single output DMA (keeps total DMA count <= 8 lanes)
            nc.sync.dma_start(
                out=out_t.rearrange("(sb p) h -> p sb h", p=P), in_=out_all
            )

    return nc


def _check_single_wait(nc):
    """Return instructions with >1 sem wait (walrus on this build rejects them)."""
    bad = []
    for fn in nc.m.functions:
        for blk in fn.blocks:
            for inst in blk.instructions:
                nm = type(inst).__name__
                if nm in ("InstDrain", "InstEventSemaphore", "InstNoOp"):
                    continue
                si = inst.sync_info
                if si is not None and len(si.on_wait) > 1:
                    bad.append(
                        (nm, inst.name, [(w.ant_name, w.wait_value) for w in si.on_wait])
                    )
    return bad


# ----------------------------------------------------------------- host prep
DW_SPLIT = (3, 0, 3)  # depthwise channel blocks on (DVE, GpSimd, PE)


def _prep_dil_inputs(x, wd, wp, gamma, beta, w_sel, c_add, K, split=None):
    """Per-core input maps for the dil_conv kernel."""
    import ml_dtypes

    bf16 = ml_dtypes.bfloat16
    split = split or DW_SPLIT
    n_pe = split[2]
    SP = S + 2 * (K - 1)
    NSB = S // P
    wd = wd[:, 0, :].astype(_f32)  # [H, K]
    wp_t = np.ascontiguousarray(wp[:, :, 0].astype(_f32).T)  # [ci, co]

    wd_pk = np.ascontiguousarray(
        wd.reshape(NB, P, K).transpose(1, 0, 2).reshape(P, NB * K)
    )
    wp_pk = np.ascontiguousarray(
        wp_t.reshape(NB, P, H).transpose(1, 0, 2).astype(bf16)
    )
    gp = (w_sel * gamma).astype(_f32)
    bp = (w_sel * beta + c_add).astype(_f32)
    gb_pk = np.ascontiguousarray(
        np.concatenate([gp.reshape(NB, P).T, bp.reshape(NB, P).T], axis=1)
    ).astype(_f32)
    wg_pk = np.ascontiguousarray(np.concatenate([wd_pk, gb_pk], axis=1))

    # diagonalized taps for the PE depthwise blocks (last n_pe blocks)
    wdiag = np.zeros((P, max(n_pe, 1), K, P), _f32)
    for i in range(n_pe):
        b = NB - n_pe + i
        blk = wd.reshape(NB, P, K)[b]  # [P, K]
        for j in range(K):
            np.fill_diagonal(wdiag[:, i, j, :], blk[:, j])
    wdiag = wdiag.astype(bf16)

    in_maps = []
    for c in range(N_CORES):
        xb = x[c].astype(_f32)  # [S, H]
        xr = np.maximum(xb, 0.0).T  # [H, S] relu'd
        xr_pad = np.zeros((H, SP), _f32)
        xr_pad[:, K - 1 : K - 1 + S] = xr
        xr_pk = np.ascontiguousarray(
            xr_pad.reshape(NB, P, SP).transpose(1, 0, 2).astype(bf16)
        )
        xn_pk = np.ascontiguousarray(xb.reshape(NSB, P, H).transpose(1, 0, 2))
        in_maps.append(
            {
                "xr_pad": xr_pk,
                "x_nat": xn_pk,
                "wg_pk": wg_pk,
                "wp_t": wp_pk,
                "wdiag": wdiag,
            }
        )
    return in_maps


def _run_dil(x, wd, wp, gamma, beta, w_sel, c_add, K):
    from concourse.bass_utils import run_bass_kernel_spmd

    key = ("dil", K, DW_SPLIT)
    if key not in _BUILD_CACHE:
        nc = _build_dil_kernel(K, split=DW_SPLIT)
        bad = _check_single_wait(nc)
        if bad:
            raise RuntimeError(f"multi-wait instructions would fail codegen: {bad}")
        _BUILD_CACHE[key] = nc
    nc = _BUILD_CACHE[key]
    in_maps = _prep_dil_inputs(x, wd, wp, gamma, beta, w_sel, c_add, K, DW_SPLIT)
    res = run_bass_kernel_spmd(nc, in_maps, core_ids=list(range(N_CORES)))
    out = np.stack([res.results[c]["out"] for c in range(N_CORES)], axis=0)
    return out


# ------------------------------------------------- host fallbacks (non-conv)
def _branch_host(idx, x, inputs):
    """Numpy fallback for the trivial branches (never selected with the
    benchmark inputs; kept for completeness)."""
    xc = np.transpose(x, (0, 2, 1)).astype(_f32)  # [B, H, S]
    if idx == 0:
        return np.zeros_like(xc)
    if idx == 1:
        xp = np.pad(xc, ((0, 0), (0, 0), (1, 1)))
        return (xp[:, :, :-2] + xp[:, :, 1:-1] + xp[:, :, 2:]) / _f32(3.0)
    if idx == 2:
        xp = np.pad(xc, ((0, 0), (0, 0), (1, 1)), constant_values=-np.inf)
        return np.maximum(np.maximum(xp[:, :, :-2], xp[:, :, 1:-1]), xp[:, :, 2:])
    if idx == 9:
        return xc
    raise AssertionError(idx)


def _bn_host(y, gamma, beta):
    m = y.mean(axis=(0, 2), keepdims=True)
    v = y.var(axis=(0, 2), keepdims=True)
    return (y - m) / np.sqrt(v + EPS) * gamma[None, :, None] + beta[None, :, None]


def _nor_conv_host(x, w, gamma, beta, k):
    xc = np.transpose(x, (0, 2, 1)).astype(_f32)
    xr = np.maximum(xc, 0.0)
    pad = k // 2
    xp = np.pad(xr, ((0, 0), (0, 0), (pad, pad)))
    y = np.zeros((B, H, S), _f32)
    for j in range(k):
        y += np.einsum("oi,bis->bos", w[:, :, j], xp[:, :, j : j + S], optimize=True)
    return _bn_host(y, gamma, beta)


# ------------------------------------------------------------------- kernel
def kernel(**inputs):
    x = np.asarray(inputs["x"], dtype=_f32)
    idx, w_sel, c_add = _gate(
        np.asarray(inputs["u"]), np.asarray(inputs["arch_parameters"])
    )

    if idx in (6, 7, 8):
        K = {6: 3, 7: 5, 8: 7}[idx]
        out = _run_dil(
            x,
            np.asarray(inputs[f"wd_dil{K}"]),
            np.asarray(inputs[f"wp_dil{K}"]),
            np.asarray(inputs[f"g_dil{K}"], dtype=_f32),
            np.asarray(inputs[f"b_dil{K}"], dtype=_f32),
            w_sel,
            c_add,
            K,
        )
        return out.astype(_f32)

    # Branches that are never selected with the benchmark gate inputs: host math.
    if idx in (3, 4, 5):
        k = {3: 3, 4: 5, 5: 7}[idx]
        sel = _nor_conv_host(
            x, np.asarray(inputs[f"w_nor{k}"], dtype=_f32),
            np.asarray(inputs[f"g_nor{k}"], dtype=_f32),
            np.asarray(inputs[f"b_nor{k}"], dtype=_f32), k,
        )
    else:
        sel = _branch_host(idx, x, inputs)
    out = w_sel * sel + c_add
    out = np.transpose(out, (0, 2, 1))
    return (out + x).astype(_f32)
